# revision 28
# baseline (speedup 1.0000x reference)
"""Multi-head attention (S=2048, D=1024, H=16, dk=dv=64) on 8 TRN2 NeuronCores.

Head-parallel tensor parallelism: core c owns heads {2c, 2c+1}.
All operands stream in bf16 (host-cast); fp32 PSUM accumulation; softmax
denominators via a ones-column folded into the ctx matmul lhsT (V_aug).

Host-side packing lays every enc tensor out in exactly the SBUF tile
order so each DMA is a contiguous >=8KB-per-partition read.

Schedule (s processed in 4 chunks of 512 queries):
  warm-up: PE burst + tiny dummy AllGather (absorbs the ~11us
           first-collective init on the CC core).
  chunk 0: kacc groups at tt 0/3/6/9 through a 2-buf PSUM ring
           (interleaved with that chunk's scores so scores(0,tt) start
           as soon as K tile tt//4 is projected); qq1 at tt0-7; V proj
           4 mm/tt at tt8-15; transposes 0-7 + qq23 at tt12-15.
  chunk 1: transposes 8-15 + ctx(0) catch-up at 2 k/tt (tt0-7);
           normalize(0) at tt8; ctx(1) lag-8.
  chunk 2: ctx(1) drain, normalize(1) -> AllGather(chunks 0+1) at tt8,
           ctx(2) lag 8.
  chunk 3: ctx(2) drain, normalize(2) -> AllGather(chunk 2) at tt8,
           ctx(3) lag 8.
  tail: ctx(3) drain interleaved with outproj(0,1) (gathered data
           arrives mid-chunk-3), normalize(3) -> AllGather(chunk 3),
           outproj(2) (AG2 data), outproj(3), output DMAs.
"""

import numpy as np
import ml_dtypes

import concourse.bass as bass
import concourse.mybir as mybir
import concourse.tile as tile
from concourse import bacc
from concourse.bass_utils import run_bass_kernel_spmd

S = 2048
D = 1024
H = 16
DK = 64
DV = 64
NCORES = 8
HPC = H // NCORES          # heads per core = 2
FW = HPC * DV              # per-core feature width = 128
P = 128                    # partitions
KT_D = D // P              # 8 contraction tiles over D
TT = S // P                # 16 tiles over t (keys)
NQ = 512                   # scores matmul free dim (per head)
CW = 512                   # s-chunk width
NCH = S // CW              # 4 chunks
VA = 2 * (DV + 1)          # V_aug feature width (v0,one0,v1,one1)

F32 = mybir.dt.float32
BF16 = mybir.dt.bfloat16
EXPF = mybir.ActivationFunctionType.Exp

_cache = {}


def _prep_w(w):
    """[D, FW] -> [128, KT_D*FW] bf16: row p holds all d-tiles' row p."""
    t = w.reshape(KT_D, P, FW)
    return np.ascontiguousarray(
        np.transpose(t, (1, 0, 2)).reshape(P, KT_D * FW)
    ).astype(ml_dtypes.bfloat16)


def build():
    nc = bacc.Bacc(None, target_bir_lowering=False)

    ekp_in = nc.dram_tensor("ekp", [P, 4 * KT_D * NQ], BF16,
                            kind="ExternalInput")
    eqp_in = nc.dram_tensor("eqp", [P, NCH * KT_D * CW], BF16,
                            kind="ExternalInput")
    evp_in = nc.dram_tensor("evp", [P, 2 * 4 * S], BF16,
                            kind="ExternalInput")
    w_in = {
        n: nc.dram_tensor(n, [P, KT_D * FW], BF16, kind="ExternalInput")
        for n in ("wq", "wk", "wv", "wo")
    }
    out_t = nc.dram_tensor("outT", [FW, S], F32, kind="ExternalOutput")

    from concourse.masks import make_identity

    with tile.TileContext(nc) as tc:
        with (
            tc.tile_pool(name="wts", bufs=1) as wts,
            tc.tile_pool(name="encp", bufs=1) as encp,
            tc.tile_pool(name="qkv", bufs=1) as qkv,
            tc.tile_pool(name="expp", bufs=16) as expp,
            tc.tile_pool(name="catp", bufs=1) as catp,
            tc.tile_pool(name="catin", bufs=1) as catin,
            tc.tile_pool(name="misc", bufs=1) as misc,
            tc.tile_pool(name="dram", bufs=1, space="DRAM") as dram,
        ):
            rg = [list(range(NCORES))]

            wtiles = {
                n: wts.tile([P, KT_D, FW], BF16, tag=f"w_{n}", name=n)
                for n in ("wq", "wk", "wv", "wo")
            }

            def wload(n, eng):
                eng.dma_start(
                    wtiles[n].rearrange("p kt m -> p (kt m)"), w_in[n][:]
                )

            ident = wts.tile([P, P], BF16, tag="ident")
            make_identity(nc, ident)

            # persistent SBUF state
            qt_sb = qkv.tile([P, S], BF16, tag="qt")
            kt_sb = qkv.tile([P, S], BF16, tag="kt")
            vt_sb = qkv.tile([P, S], BF16, tag="vt")
            v_aug = qkv.tile([P, TT, VA], BF16, tag="vaug")
            cat_loc = catp.tile([P, S], BF16, tag="cat")
            # head-select matrix for the normalize broadcast matmul:
            # row 0 selects head-0's 64-partition block, row 32 head-1's.
            sel2 = misc.tile([33, P], F32, tag="sel2", bufs=1, name="sel2")
            nc.any.memset(sel2[:], 0.0)
            nc.any.memset(sel2[0:1, 0:DV], 1.0)
            nc.any.memset(sel2[32:33, DV : 2 * DV], 1.0)
            nc.any.memset(v_aug[:, :, DV : DV + 1], 1.0)
            nc.any.memset(v_aug[:, :, 2 * DV + 1 : 2 * DV + 2], 1.0)

            # ---- enc tiles ----
            ek_tiles = [
                encp.tile([P, KT_D, NQ], BF16, tag="ek", bufs=4, name="ek")
                for _ in range(4)
            ]
            eq_tiles = {
                ci: encp.tile([P, KT_D, CW], BF16, tag="eq", bufs=4,
                              name="eq")
                for ci in range(4)
            }
            ev_tiles = [
                encp.tile([P, 4, S], BF16, tag="ev", bufs=2, name="ev")
                for _ in range(2)
            ]

            def ekload(sc4, eng):
                eng.dma_start(
                    ek_tiles[sc4].rearrange("p kt m -> p (kt m)"),
                    ekp_in[:, sc4 * KT_D * NQ : (sc4 + 1) * KT_D * NQ],
                )

            def eqload(ci, eng):
                eng.dma_start(
                    eq_tiles[ci].rearrange("p kt m -> p (kt m)"),
                    eqp_in[:, ci * KT_D * CW : (ci + 1) * KT_D * CW],
                )

            def evload(half, eng):
                eng.dma_start(
                    ev_tiles[half].rearrange("p d s -> p (d s)"),
                    evp_in[:, half * 4 * S : (half + 1) * 4 * S],
                )

            # need-ordered, balanced across the two HWDGE queues
            ekload(0, nc.sync)
            eqload(0, nc.scalar)
            wload("wk", nc.sync)
            wload("wq", nc.scalar)
            ekload(2, nc.sync)
            ekload(1, nc.scalar)
            eqload(1, nc.sync)
            ekload(3, nc.scalar)
            wload("wv", nc.scalar)
            evload(0, nc.sync)
            evload(1, nc.scalar)
            eqload(3, nc.sync)
            eqload(2, nc.scalar)
            wload("wo", nc.scalar)

            # ---- dummy warm-up collective: absorbs the one-time CC
            # init (~11us) while the enc DMAs stream. Output unused.
            dumb_in = dram.tile([P, 32], BF16, tag="dumb_i", name="di")
            dumb_out = dram.tile([D, 32], BF16, tag="dumb_o", name="do",
                                 addr_space="Shared")
            nc.gpsimd.collective_compute(
                "AllGather",
                mybir.AluOpType.bypass,
                ins=[dumb_in[:].opt()],
                outs=[dumb_out[:].opt()],
                replica_groups=rg,
            )

            def ev(dt):
                return ev_tiles[dt // 4][:, dt % 4, :]

            # ---- chunk 0 PSUM pools ----
            # 2-buf ring shared by warmup burst, qq0 and the kacc groups
            ps_ring_cm = tc.tile_pool(name="ps_ring", bufs=2, space="PSUM")
            ps_ring = ps_ring_cm.__enter__()
            # warmup burst: lhsT/rhs are uninitialized SBUF on purpose —
            # zero input deps so Tile schedules these at the head of the
            # PE queue (values are discarded; the ring slot is cleared by
            # the next start=True group).
            wm = ps_ring.tile([P, NQ], F32, tag="ka", name="wm")
            for _ in range(14):
                nc.tensor.matmul(wm[:], vt_sb[:, 0:P], vt_sb[:, 0:NQ],
                                 start=True, stop=True)
            qq0 = ps_ring.tile([P, CW], F32, tag="ka", name="qq0")
            for dt in range(KT_D):
                nc.tensor.matmul(
                    qq0[:], wtiles["wq"][:, dt, :], eq_tiles[0][:, dt, :],
                    start=(dt == 0), stop=(dt == KT_D - 1),
                )
            nc.vector.tensor_copy(qt_sb[:, 0:CW], qq0[:])

            ps_m0_cm = tc.tile_pool(name="ps_m0", bufs=1, space="PSUM")
            ps_m0 = ps_m0_cm.__enter__()
            ps_q1_cm = tc.tile_pool(name="ps_q1", bufs=1, space="PSUM")
            ps_q1 = ps_q1_cm.__enter__()
            qq1 = ps_q1.tile([P, CW], F32, tag="qq1", name="qq1")

            # ---- attention helpers ----
            exs = {}

            def scores_tt(ci, tt, pool, bufs):
                m = pool.tile([P, 1024], F32, tag="mega", bufs=bufs,
                              name="m")
                s0 = ci * CW
                for h in range(HPC):
                    nc.tensor.matmul(
                        m[:, h * NQ : (h + 1) * NQ],
                        kt_sb[h * DK : (h + 1) * DK, tt * P : (tt + 1) * P],
                        qt_sb[h * DK : (h + 1) * DK, s0 : s0 + NQ],
                        start=True,
                        stop=True,
                    )
                ex = expp.tile(
                    [P, 1024], BF16, tag="exp", bufs=16, name="ex"
                )
                nc.scalar.activation(ex[:], m[:], EXPF, scale=1.0 / np.sqrt(DK))
                exs[(ci, tt)] = ex

            ctx_ps = {}

            def ctx_op(ci, k):
                ex = exs.pop((ci, k))
                for h in range(HPC):
                    nc.tensor.matmul(
                        ctx_ps[h][:],
                        v_aug[:, k, h * (DV + 1) : (h + 1) * (DV + 1)],
                        ex[:, h * NQ : (h + 1) * NQ],
                        start=(k == 0),
                        stop=(k == TT - 1),
                    )

            def transp(k):
                tp = tp_t[:, k % 2, :]
                nc.tensor.transpose(tp, vt_sb[:, k * P : (k + 1) * P], ident[:])
                nc.vector.tensor_copy(v_aug[:, k, 0:DV], tp[:, 0:DV])
                nc.vector.tensor_copy(
                    v_aug[:, k, DV + 1 : 2 * DV + 1], tp[:, DV : 2 * DV]
                )



            def normalize(ci):
                c0 = ci * CW
                den2 = misc.tile([33, CW], F32, tag="den2", bufs=2,
                                 name="den2")
                nc.vector.memset(den2[:], 1.0)
                nc.vector.tensor_copy(den2[0:1, :], ctx_ps[0][DV : DV + 1, :])
                nc.vector.tensor_copy(den2[32:33, :],
                                      ctx_ps[1][DV : DV + 1, :])
                recip2 = misc.tile([33, CW], F32, tag="recip2", bufs=2,
                                   name="recip2")
                nc.vector.reciprocal_approx_fast(recip2[:], den2[:])
                bc_ps = ps_op.tile([P, CW], F32, tag="bc", bufs=1, name="bc")
                nc.tensor.matmul(bc_ps[:], sel2[:], recip2[:],
                                 start=True, stop=True)
                bcast = misc.tile([P, CW], F32, tag="bcast", bufs=2,
                                  name="bcast")
                nc.vector.tensor_copy(bcast[:], bc_ps[:])
                for h in range(HPC):
                    nc.vector.tensor_mul(
                        cat_loc[h * DV : (h + 1) * DV, c0 : c0 + CW],
                        ctx_ps[h][0:DV, :],
                        bcast[h * DV : (h + 1) * DV, :],
                    )

            # Per-chunk AllGather, split into stage (cb DMA on Sync +
            # trigger on GpSimd, fires right at normalize) and fetch
            # (catin DMAs on GpSimd SWDGE — their long wait for the
            # collective cannot block the Sync/Scalar queues).
            ga_t = {}
            cat_sb = {}

            def cc_stage(ci):
                c0 = ci * CW
                cb = dram.tile([P, CW], BF16, tag=f"catb{ci}", name="cb")
                nc.sync.dma_start(cb[:], cat_loc[:, c0 : c0 + CW])
                ga = dram.tile([D, CW], BF16, tag=f"catall{ci}", name="ga",
                               addr_space="Shared")
                nc.gpsimd.collective_compute(
                    "AllGather",
                    mybir.AluOpType.bypass,
                    ins=[cb[:].opt()],
                    outs=[ga[:].opt()],
                    replica_groups=rg,
                )
                ga_t[ci] = ga

            def cc_fetch(ci):
                # Scalar HWDGE: idle after the last exp, and nothing is
                # queued behind these, so their wait on the collective's
                # completion blocks nothing.
                t = catin.tile([P, KT_D, CW], BF16, tag=f"ci{ci}", bufs=1,
                               name="ct")
                gav = ga_t[ci][:].rearrange("(kt p) s -> p kt s", kt=KT_D)
                nc.scalar.dma_start(t[:, 0:4, :], gav[:, 0:4, :])
                nc.scalar.dma_start(t[:, 4:8, :], gav[:, 4:8, :])
                cat_sb[ci] = t

            opm = {}

            def outproj_mm(ci, kt):
                nc.tensor.matmul(
                    opm[ci][:],
                    wtiles["wo"][:, kt, :],
                    cat_sb[ci][:, kt, :],
                    start=(kt == 0),
                    stop=(kt == KT_D - 1),
                )

            def outproj_start(ci):
                opm[ci] = ps_opm.tile([P, CW], F32, tag="opm", bufs=2,
                                      name=f"opm{ci}")

            def outproj_store(ci):
                c0 = ci * CW
                ob = misc.tile([P, CW], F32, tag="ob", bufs=2, name="ob")
                nc.vector.tensor_copy(ob[:], opm[ci][:])
                nc.sync.dma_start(out_t[:, c0 : c0 + CW], ob[:])

            # ================= chunk 0 =================
            # kacc groups through the ring at tt 0/3/6/9; scores follow
            # each projected K block. V proj 4mm/tt at tt8-15.
            kacc = {}
            for tt in range(TT):
                if tt in (0, 3, 6, 9):
                    sc4 = tt // 3 if tt else 0
                    ka = ps_ring.tile([P, NQ], F32, tag="ka",
                                      name=f"ka{sc4}")
                    for dt in range(KT_D):
                        nc.tensor.matmul(
                            ka[:],
                            wtiles["wk"][:, dt, :],
                            ek_tiles[sc4][:, dt, :],
                            start=(dt == 0),
                            stop=(dt == KT_D - 1),
                        )
                    nc.vector.tensor_copy(
                        kt_sb[:, sc4 * NQ : (sc4 + 1) * NQ], ka[:]
                    )
                scores_tt(0, tt, ps_m0, 1)
                if tt < 8:
                    nc.tensor.matmul(
                        qq1[:], wtiles["wq"][:, tt, :],
                        eq_tiles[1][:, tt, :],
                        start=(tt == 0), stop=(tt == KT_D - 1),
                    )
                else:
                    j = tt - 8
                    if tt == 8:
                        nc.vector.tensor_copy(qt_sb[:, CW : 2 * CW], qq1[:])
                        ps_q1_cm.__exit__(None, None, None)
                        ps_v_cm = tc.tile_pool(name="ps_v", bufs=1,
                                               space="PSUM")
                        ps_v = ps_v_cm.__enter__()
                        vacc = ps_v.tile([P, 1024], F32, tag="vacc",
                                         name="vacc")
                    if tt == 12:
                        nc.vector.tensor_copy(vt_sb[:, 0:1024], vacc[:])
                        ps_tp0_cm = tc.tile_pool(name="ps_tp0", bufs=1,
                                                 space="PSUM")
                        ps_tp0 = ps_tp0_cm.__enter__()
                        tp_t = ps_tp0.tile([P, 2, P], BF16, tag="tp",
                                           name="tp0")
                    half, jj = divmod(j, 4)
                    for dt in (2 * jj, 2 * jj + 1):
                        for nn in range(2):
                            nc.tensor.matmul(
                                vacc[:, nn * NQ : (nn + 1) * NQ],
                                wtiles["wv"][:, dt, :],
                                ev(dt)[:, half * 1024 + nn * NQ :
                                       half * 1024 + (nn + 1) * NQ],
                                start=(dt == 0),
                                stop=(dt == KT_D - 1),
                            )
                    if tt >= 12:
                        transp(2 * (tt - 12))
                        transp(2 * (tt - 12) + 1)

            # ================= chunk 1 =================
            nc.vector.tensor_copy(vt_sb[:, 1024:2048], vacc[:])
            ps_tp0_cm.__exit__(None, None, None)
            ps_v_cm.__exit__(None, None, None)
            ps_m0_cm.__exit__(None, None, None)
            ps_ring_cm.__exit__(None, None, None)

            # pool stack: cx (outer), op, mega (closes first, at tail)
            ps_cx_cm = tc.tile_pool(name="ps_cx", bufs=1, space="PSUM")
            ps_cx = ps_cx_cm.__enter__()
            for h in range(HPC):
                ctx_ps[h] = ps_cx.tile(
                    [DV + 1, CW], F32, tag=f"cx{h}", name=f"cx{h}"
                )
            ps_op_cm = tc.tile_pool(name="ps_op", bufs=1, space="PSUM")
            ps_op = ps_op_cm.__enter__()
            ps_mega_cm = tc.tile_pool(name="ps_mega", bufs=1, space="PSUM")
            ps_mega = ps_mega_cm.__enter__()
            ps_tp1_cm = tc.tile_pool(name="ps_tp1", bufs=1, space="PSUM")
            ps_tp1 = ps_tp1_cm.__enter__()
            tp_t = ps_tp1.tile([P, 2, P], BF16, tag="tp1", name="tp1")

            for tt in range(TT):
                scores_tt(1, tt, ps_mega, 2)
                if tt < 8:
                    transp(8 + tt)
                    ctx_op(0, 2 * tt)
                    ctx_op(0, 2 * tt + 1)
                else:
                    if tt == 8:
                        ps_tp1_cm.__exit__(None, None, None)
                        ps_q23_cm = tc.tile_pool(name="ps_q23", bufs=1,
                                                 space="PSUM")
                        ps_q23 = ps_q23_cm.__enter__()
                        qq23 = ps_q23.tile([P, CW], F32, tag="qq23",
                                           name="qq23")
                        normalize(0)
                        cc_stage(0)
                    ctx_op(1, tt - 8)
                    # q2 proj at tt 8..11 (2 d-tiles/tt), q3 at 12..15
                    j = tt - 8
                    qi, j4 = (2, j) if j < 4 else (3, j - 4)
                    for dt2 in (2 * j4, 2 * j4 + 1):
                        nc.tensor.matmul(
                            qq23[:], wtiles["wq"][:, dt2, :],
                            eq_tiles[qi][:, dt2, :],
                            start=(dt2 == 0), stop=(dt2 == KT_D - 1),
                        )
                    if tt == 11:
                        nc.vector.tensor_copy(
                            qt_sb[:, 2 * CW : 3 * CW], qq23[:]
                        )

            # ================= chunk 2 =================
            nc.vector.tensor_copy(qt_sb[:, 3 * CW : 4 * CW], qq23[:])
            ps_q23_cm.__exit__(None, None, None)
            for tt in range(TT):
                scores_tt(2, tt, ps_mega, 2)
                if tt < 8:
                    ctx_op(1, tt + 8)
                else:
                    if tt == 8:
                        normalize(1)
                        cc_stage(1)
                    ctx_op(2, tt - 8)

            # ================= chunk 3 =================
            # ctx(3) at lag 2 (2 k/tt) so normalize(3) follows the last
            # scores almost immediately.
            for tt in range(TT):
                scores_tt(3, tt, ps_mega, 2)
                if tt < 8:
                    ctx_op(2, tt + 8)
                else:
                    if tt == 8:
                        normalize(2)
                        cc_stage(2)
                    ctx_op(3, 2 * (tt - 8))
                    ctx_op(3, 2 * (tt - 8) + 1)

            # ================= tail =================
            ps_mega_cm.__exit__(None, None, None)
            ps_opm_cm = tc.tile_pool(name="ps_opm", bufs=1, space="PSUM")
            ps_opm = ps_opm_cm.__enter__()
            normalize(3)
            cc_stage(3)
            # catin fetches on GpSimd strictly after every trigger; each
            # waits only for its own collective's completion.
            for ci in range(4):
                cc_fetch(ci)
            # op0/op1 (data long since landed) fill the AG2/AG3 mesh
            # windows; op2/op3 follow their fetches.
            for ci in range(4):
                outproj_start(ci)
                for kt in range(KT_D):
                    outproj_mm(ci, kt)
                outproj_store(ci)

            ps_opm_cm.__exit__(None, None, None)
            ps_op_cm.__exit__(None, None, None)
            ps_cx_cm.__exit__(None, None, None)

    nc.compile()
    return nc


def kernel(
    encodings_for_q,
    encodings_for_k,
    encodings_for_v,
    W_q,
    W_k,
    W_v,
    W_out,
    _trace: bool = False,
):
    encodings_for_q = np.asarray(encodings_for_q, dtype=np.float32)
    encodings_for_k = np.asarray(encodings_for_k, dtype=np.float32)
    encodings_for_v = np.asarray(encodings_for_v, dtype=np.float32)
    W_q = np.asarray(W_q, dtype=np.float32)
    W_k = np.asarray(W_k, dtype=np.float32)
    W_v = np.asarray(W_v, dtype=np.float32)
    W_out = np.asarray(W_out, dtype=np.float32)

    if "nc" not in _cache:
        _cache["nc"] = build()
    nc = _cache["nc"]

    eqT = np.ascontiguousarray(encodings_for_q.T).astype(ml_dtypes.bfloat16)
    ekT = np.ascontiguousarray(encodings_for_k.T).astype(ml_dtypes.bfloat16)
    evT = np.ascontiguousarray(encodings_for_v.T).astype(ml_dtypes.bfloat16)

    # pack in SBUF tile order (contiguous per-partition DMA slices):
    # ekp[p, sc4, kt, m], eqp[p, ci, kt, m], evp[p, half, dt, s]
    ekp = np.ascontiguousarray(
        ekT.reshape(KT_D, P, 4, NQ).transpose(1, 2, 0, 3).reshape(P, -1)
    )
    eqp = np.ascontiguousarray(
        eqT.reshape(KT_D, P, NCH, CW).transpose(1, 2, 0, 3).reshape(P, -1)
    )
    evp = np.ascontiguousarray(
        evT.reshape(2, 4, P, S).transpose(2, 0, 1, 3).reshape(P, -1)
    )

    in_maps = []
    for c in range(NCORES):
        hs = slice(HPC * c, HPC * (c + 1))
        in_maps.append(
            {
                "ekp": ekp,
                "eqp": eqp,
                "evp": evp,
                "wq": _prep_w(np.transpose(W_q[hs], (1, 0, 2)).reshape(D, FW)),
                "wk": _prep_w(np.transpose(W_k[hs], (1, 0, 2)).reshape(D, FW)),
                "wv": _prep_w(np.transpose(W_v[hs], (1, 0, 2)).reshape(D, FW)),
                "wo": _prep_w(W_out[:, FW * c : FW * (c + 1)]),
            }
        )

    r = run_bass_kernel_spmd(
        nc, in_maps, core_ids=list(range(NCORES)), trace=_trace
    )
    out = np.concatenate(
        [r.results[c]["outT"].T for c in range(NCORES)], axis=1
    )
    if _trace:
        kernel.last_exec_time_ns = r.exec_time_ns
        kernel.last_insts = (
            r.instructions_and_trace[0] if r.instructions_and_trace else None
        )
    return out.astype(np.float32)


# revision 29
# speedup vs baseline: 1.0333x; 1.0333x over previous
"""Multi-head attention (S=2048, D=1024, H=16, dk=dv=64) on 8 TRN2 NeuronCores.

Head-parallel tensor parallelism: core c owns heads {2c, 2c+1}.
All operands stream in bf16 (host-cast); fp32 PSUM accumulation; softmax
denominators via a ones-column folded into the ctx matmul lhsT (V_aug).

Host-side packing lays every enc tensor out in exactly the SBUF tile
order so each DMA is a contiguous >=8KB-per-partition read.

Schedule (s processed in 4 chunks of 512 queries):
  warm-up: PE burst + tiny dummy AllGather (absorbs the ~11us
           first-collective init on the CC core).
  chunk 0: kacc groups at tt 0/3/6/9 through a 2-buf PSUM ring
           (interleaved with that chunk's scores so scores(0,tt) start
           as soon as K tile tt//4 is projected); qq1 at tt0-7; V proj
           4 mm/tt at tt8-15; transposes 0-7 + qq23 at tt12-15.
  chunk 1: transposes 8-15 + ctx(0) catch-up at 2 k/tt (tt0-7);
           normalize(0) at tt8; ctx(1) lag-8.
  chunk 2: ctx(1) drain, normalize(1) -> AllGather(chunks 0+1) at tt8,
           ctx(2) lag 8.
  chunk 3: ctx(2) drain, normalize(2) -> AllGather(chunk 2) at tt8,
           ctx(3) lag 8.
  tail: ctx(3) drain interleaved with outproj(0,1) (gathered data
           arrives mid-chunk-3), normalize(3) -> AllGather(chunk 3),
           outproj(2) (AG2 data), outproj(3), output DMAs.
"""

import numpy as np
import ml_dtypes

import concourse.bass as bass
import concourse.mybir as mybir
import concourse.tile as tile
from concourse import bacc
from concourse.bass_utils import run_bass_kernel_spmd

S = 2048
D = 1024
H = 16
DK = 64
DV = 64
NCORES = 8
HPC = H // NCORES          # heads per core = 2
FW = HPC * DV              # per-core feature width = 128
P = 128                    # partitions
KT_D = D // P              # 8 contraction tiles over D
TT = S // P                # 16 tiles over t (keys)
NQ = 512                   # scores matmul free dim (per head)
CW = 512                   # s-chunk width
NCH = S // CW              # 4 chunks
VA = 2 * (DV + 1)          # V_aug feature width (v0,one0,v1,one1)

F32 = mybir.dt.float32
BF16 = mybir.dt.bfloat16
EXPF = mybir.ActivationFunctionType.Exp

_cache = {}


def _prep_w(w):
    """[D, FW] -> [128, KT_D*FW] bf16: row p holds all d-tiles' row p."""
    t = w.reshape(KT_D, P, FW)
    return np.ascontiguousarray(
        np.transpose(t, (1, 0, 2)).reshape(P, KT_D * FW)
    ).astype(ml_dtypes.bfloat16)


def build():
    nc = bacc.Bacc(None, target_bir_lowering=False)

    ekp_in = nc.dram_tensor("ekp", [P, 4 * KT_D * NQ], BF16,
                            kind="ExternalInput")
    eqp_in = nc.dram_tensor("eqp", [P, NCH * KT_D * CW], BF16,
                            kind="ExternalInput")
    evp_in = nc.dram_tensor("evp", [P, 2 * 4 * S], BF16,
                            kind="ExternalInput")
    w_in = {
        n: nc.dram_tensor(n, [P, KT_D * FW], BF16, kind="ExternalInput")
        for n in ("wq", "wk", "wv", "wo")
    }
    out_t = nc.dram_tensor("outT", [FW, S], F32, kind="ExternalOutput")

    from concourse.masks import make_identity

    with tile.TileContext(nc) as tc:
        with (
            tc.tile_pool(name="wts", bufs=1) as wts,
            tc.tile_pool(name="encp", bufs=1) as encp,
            tc.tile_pool(name="qkv", bufs=1) as qkv,
            tc.tile_pool(name="expp", bufs=16) as expp,
            tc.tile_pool(name="catp", bufs=1) as catp,
            tc.tile_pool(name="catin", bufs=1) as catin,
            tc.tile_pool(name="misc", bufs=1) as misc,
            tc.tile_pool(name="dram", bufs=1, space="DRAM") as dram,
        ):
            rg = [list(range(NCORES))]

            wtiles = {
                n: wts.tile([P, KT_D, FW], BF16, tag=f"w_{n}", name=n)
                for n in ("wq", "wk", "wv", "wo")
            }

            def wload(n, eng):
                eng.dma_start(
                    wtiles[n].rearrange("p kt m -> p (kt m)"), w_in[n][:]
                )

            ident = wts.tile([P, P], BF16, tag="ident")
            make_identity(nc, ident)

            # persistent SBUF state
            qt_sb = qkv.tile([P, S], BF16, tag="qt")
            kt_sb = qkv.tile([P, S], BF16, tag="kt")
            vt_sb = qkv.tile([P, S], BF16, tag="vt")
            v_aug = qkv.tile([P, TT, VA], BF16, tag="vaug")
            cat_loc = catp.tile([P, S], BF16, tag="cat")
            # head-select matrix for the normalize broadcast matmul:
            # row 0 selects head-0's 64-partition block, row 32 head-1's.
            sel2 = misc.tile([33, P], F32, tag="sel2", bufs=1, name="sel2")
            nc.any.memset(sel2[:], 0.0)
            nc.any.memset(sel2[0:1, 0:DV], 1.0)
            nc.any.memset(sel2[32:33, DV : 2 * DV], 1.0)
            nc.any.memset(v_aug[:, :, DV : DV + 1], 1.0)
            nc.any.memset(v_aug[:, :, 2 * DV + 1 : 2 * DV + 2], 1.0)

            # ---- enc tiles ----
            ek_tiles = [
                encp.tile([P, KT_D, NQ], BF16, tag="ek", bufs=4, name="ek")
                for _ in range(4)
            ]
            eq_tiles = {
                ci: encp.tile([P, KT_D, CW], BF16, tag="eq", bufs=4,
                              name="eq")
                for ci in range(4)
            }
            ev_tiles = [
                encp.tile([P, 4, S], BF16, tag="ev", bufs=2, name="ev")
                for _ in range(2)
            ]

            def ekload(sc4, eng):
                eng.dma_start(
                    ek_tiles[sc4].rearrange("p kt m -> p (kt m)"),
                    ekp_in[:, sc4 * KT_D * NQ : (sc4 + 1) * KT_D * NQ],
                )

            def eqload(ci, eng):
                eng.dma_start(
                    eq_tiles[ci].rearrange("p kt m -> p (kt m)"),
                    eqp_in[:, ci * KT_D * CW : (ci + 1) * KT_D * CW],
                )

            def evload(half, eng):
                eng.dma_start(
                    ev_tiles[half].rearrange("p d s -> p (d s)"),
                    evp_in[:, half * 4 * S : (half + 1) * 4 * S],
                )

            # need-ordered, balanced across the two HWDGE queues
            ekload(0, nc.sync)
            eqload(0, nc.scalar)
            wload("wk", nc.sync)
            wload("wq", nc.scalar)
            ekload(2, nc.sync)
            ekload(1, nc.scalar)
            eqload(1, nc.sync)
            ekload(3, nc.scalar)
            wload("wv", nc.scalar)
            evload(0, nc.sync)
            evload(1, nc.scalar)
            eqload(3, nc.sync)
            eqload(2, nc.scalar)
            wload("wo", nc.scalar)

            # ---- dummy warm-up collective: absorbs the one-time CC
            # init (~11us) while the enc DMAs stream. Output unused.
            dumb_in = dram.tile([P, 32], BF16, tag="dumb_i", name="di")
            dumb_out = dram.tile([D, 32], BF16, tag="dumb_o", name="do",
                                 addr_space="Shared")
            nc.gpsimd.collective_compute(
                "AllGather",
                mybir.AluOpType.bypass,
                ins=[dumb_in[:].opt()],
                outs=[dumb_out[:].opt()],
                replica_groups=rg,
            )

            def ev(dt):
                return ev_tiles[dt // 4][:, dt % 4, :]

            # ---- chunk 0 PSUM pools ----
            # 2-buf ring shared by warmup burst, qq0 and the kacc groups
            ps_ring_cm = tc.tile_pool(name="ps_ring", bufs=2, space="PSUM")
            ps_ring = ps_ring_cm.__enter__()
            # warmup burst: lhsT/rhs are uninitialized SBUF on purpose —
            # zero input deps so Tile schedules these at the head of the
            # PE queue (values are discarded; the ring slot is cleared by
            # the next start=True group).
            wm = ps_ring.tile([P, NQ], F32, tag="ka", name="wm")
            for _ in range(14):
                nc.tensor.matmul(wm[:], vt_sb[:, 0:P], vt_sb[:, 0:NQ],
                                 start=True, stop=True)
            qq0 = ps_ring.tile([P, CW], F32, tag="ka", name="qq0")
            for dt in range(KT_D):
                nc.tensor.matmul(
                    qq0[:], wtiles["wq"][:, dt, :], eq_tiles[0][:, dt, :],
                    start=(dt == 0), stop=(dt == KT_D - 1),
                )
            nc.vector.tensor_copy(qt_sb[:, 0:CW], qq0[:])

            ps_m0_cm = tc.tile_pool(name="ps_m0", bufs=1, space="PSUM")
            ps_m0 = ps_m0_cm.__enter__()
            ps_q1_cm = tc.tile_pool(name="ps_q1", bufs=1, space="PSUM")
            ps_q1 = ps_q1_cm.__enter__()
            qq1 = ps_q1.tile([P, CW], F32, tag="qq1", name="qq1")

            # ---- attention helpers ----
            exs = {}

            def scores_tt(ci, tt, pool, bufs):
                m = pool.tile([P, 1024], F32, tag="mega", bufs=bufs,
                              name="m")
                s0 = ci * CW
                for h in range(HPC):
                    nc.tensor.matmul(
                        m[:, h * NQ : (h + 1) * NQ],
                        kt_sb[h * DK : (h + 1) * DK, tt * P : (tt + 1) * P],
                        qt_sb[h * DK : (h + 1) * DK, s0 : s0 + NQ],
                        start=True,
                        stop=True,
                    )
                ex = expp.tile(
                    [P, 1024], BF16, tag="exp", bufs=16, name="ex"
                )
                nc.scalar.activation(ex[:], m[:], EXPF, scale=1.0 / np.sqrt(DK))
                exs[(ci, tt)] = ex

            ctx_ps = {}

            def ctx_op(ci, k):
                ex = exs.pop((ci, k))
                for h in range(HPC):
                    nc.tensor.matmul(
                        ctx_ps[h][:],
                        v_aug[:, k, h * (DV + 1) : (h + 1) * (DV + 1)],
                        ex[:, h * NQ : (h + 1) * NQ],
                        start=(k == 0),
                        stop=(k == TT - 1),
                    )

            def transp(k):
                tp = tp_t[:, k % 2, :]
                nc.tensor.transpose(tp, vt_sb[:, k * P : (k + 1) * P], ident[:])
                nc.vector.tensor_copy(v_aug[:, k, 0:DV], tp[:, 0:DV])
                nc.vector.tensor_copy(
                    v_aug[:, k, DV + 1 : 2 * DV + 1], tp[:, DV : 2 * DV]
                )



            def normalize(ci):
                c0 = ci * CW
                den2 = misc.tile([33, CW], F32, tag="den2", bufs=2,
                                 name="den2")
                nc.vector.memset(den2[:], 1.0)
                nc.vector.tensor_copy(den2[0:1, :], ctx_ps[0][DV : DV + 1, :])
                nc.vector.tensor_copy(den2[32:33, :],
                                      ctx_ps[1][DV : DV + 1, :])
                recip2 = misc.tile([33, CW], F32, tag="recip2", bufs=2,
                                   name="recip2")
                nc.vector.reciprocal_approx_fast(recip2[:], den2[:])
                bc_ps = ps_op.tile([P, CW], F32, tag="bc", bufs=1, name="bc")
                nc.tensor.matmul(bc_ps[:], sel2[:], recip2[:],
                                 start=True, stop=True)
                bcast = misc.tile([P, CW], F32, tag="bcast", bufs=2,
                                  name="bcast")
                nc.vector.tensor_copy(bcast[:], bc_ps[:])
                for h in range(HPC):
                    nc.vector.tensor_mul(
                        cat_loc[h * DV : (h + 1) * DV, c0 : c0 + CW],
                        ctx_ps[h][0:DV, :],
                        bcast[h * DV : (h + 1) * DV, :],
                    )

            # Per-chunk AllGather, split into stage (cb DMA on Sync +
            # trigger on GpSimd, fires right at normalize) and fetch
            # (catin DMAs on GpSimd SWDGE — their long wait for the
            # collective cannot block the Sync/Scalar queues).
            ga_t = {}
            cat_sb = {}

            def cc_stage(ci):
                c0 = ci * CW
                cb = dram.tile([P, CW], BF16, tag=f"catb{ci}", name="cb")
                nc.sync.dma_start(cb[:], cat_loc[:, c0 : c0 + CW])
                ga = dram.tile([D, CW], BF16, tag=f"catall{ci}", name="ga",
                               addr_space="Shared")
                nc.gpsimd.collective_compute(
                    "AllGather",
                    mybir.AluOpType.bypass,
                    ins=[cb[:].opt()],
                    outs=[ga[:].opt()],
                    replica_groups=rg,
                )
                ga_t[ci] = ga

            def cc_fetch(ci):
                t = catin.tile([P, KT_D, CW], BF16, tag=f"ci{ci}", bufs=1,
                               name="ct")
                gav = ga_t[ci][:].rearrange("(kt p) s -> p kt s", kt=KT_D)
                nc.gpsimd.dma_start(t[:, 0:4, :], gav[:, 0:4, :])
                nc.gpsimd.dma_start(t[:, 4:8, :], gav[:, 4:8, :])
                cat_sb[ci] = t

            opm = {}

            def outproj_mm(ci, kt):
                nc.tensor.matmul(
                    opm[ci][:],
                    wtiles["wo"][:, kt, :],
                    cat_sb[ci][:, kt, :],
                    start=(kt == 0),
                    stop=(kt == KT_D - 1),
                )

            def outproj_start(ci):
                opm[ci] = ps_opm.tile([P, CW], F32, tag="opm", bufs=2,
                                      name=f"opm{ci}")

            def outproj_store(ci):
                c0 = ci * CW
                ob = misc.tile([P, CW], F32, tag="ob", bufs=2, name="ob")
                nc.vector.tensor_copy(ob[:], opm[ci][:])
                nc.sync.dma_start(out_t[:, c0 : c0 + CW], ob[:])

            # ================= chunk 0 =================
            # kacc groups through the ring at tt 0/3/6/9; scores follow
            # each projected K block. V proj 4mm/tt at tt8-15.
            kacc = {}
            for tt in range(TT):
                if tt in (0, 3, 6, 9):
                    sc4 = tt // 3 if tt else 0
                    ka = ps_ring.tile([P, NQ], F32, tag="ka",
                                      name=f"ka{sc4}")
                    for dt in range(KT_D):
                        nc.tensor.matmul(
                            ka[:],
                            wtiles["wk"][:, dt, :],
                            ek_tiles[sc4][:, dt, :],
                            start=(dt == 0),
                            stop=(dt == KT_D - 1),
                        )
                    nc.vector.tensor_copy(
                        kt_sb[:, sc4 * NQ : (sc4 + 1) * NQ], ka[:]
                    )
                scores_tt(0, tt, ps_m0, 1)
                if tt < 8:
                    nc.tensor.matmul(
                        qq1[:], wtiles["wq"][:, tt, :],
                        eq_tiles[1][:, tt, :],
                        start=(tt == 0), stop=(tt == KT_D - 1),
                    )
                else:
                    j = tt - 8
                    if tt == 8:
                        nc.vector.tensor_copy(qt_sb[:, CW : 2 * CW], qq1[:])
                        ps_q1_cm.__exit__(None, None, None)
                        ps_v_cm = tc.tile_pool(name="ps_v", bufs=1,
                                               space="PSUM")
                        ps_v = ps_v_cm.__enter__()
                        vacc = ps_v.tile([P, 1024], F32, tag="vacc",
                                         name="vacc")
                        ps_q23_cm = tc.tile_pool(name="ps_q23", bufs=1,
                                                 space="PSUM")
                        ps_q23 = ps_q23_cm.__enter__()
                        qq23 = ps_q23.tile([P, CW], F32, tag="qq23",
                                           name="qq23")
                    if tt == 12:
                        nc.vector.tensor_copy(vt_sb[:, 0:1024], vacc[:])
                        ps_tp0_cm = tc.tile_pool(name="ps_tp0", bufs=1,
                                                 space="PSUM")
                        ps_tp0 = ps_tp0_cm.__enter__()
                        tp_t = ps_tp0.tile([P, 2, P], BF16, tag="tp",
                                           name="tp0")
                    half, jj = divmod(j, 4)
                    for dt in (2 * jj, 2 * jj + 1):
                        for nn in range(2):
                            nc.tensor.matmul(
                                vacc[:, nn * NQ : (nn + 1) * NQ],
                                wtiles["wv"][:, dt, :],
                                ev(dt)[:, half * 1024 + nn * NQ :
                                       half * 1024 + (nn + 1) * NQ],
                                start=(dt == 0),
                                stop=(dt == KT_D - 1),
                            )
                    # q2 proj at tt 8..11 (2 d-tiles/tt), q3 at 12..15
                    qi, j4 = (2, j) if j < 4 else (3, j - 4)
                    for dt2 in (2 * j4, 2 * j4 + 1):
                        nc.tensor.matmul(
                            qq23[:], wtiles["wq"][:, dt2, :],
                            eq_tiles[qi][:, dt2, :],
                            start=(dt2 == 0), stop=(dt2 == KT_D - 1),
                        )
                    if tt == 11:
                        nc.vector.tensor_copy(
                            qt_sb[:, 2 * CW : 3 * CW], qq23[:]
                        )
                    if tt >= 12:
                        transp(2 * (tt - 12))
                        transp(2 * (tt - 12) + 1)

            # ================= chunk 1 =================
            nc.vector.tensor_copy(qt_sb[:, 3 * CW : 4 * CW], qq23[:])
            nc.vector.tensor_copy(vt_sb[:, 1024:2048], vacc[:])
            ps_tp0_cm.__exit__(None, None, None)
            ps_q23_cm.__exit__(None, None, None)
            ps_v_cm.__exit__(None, None, None)
            ps_m0_cm.__exit__(None, None, None)
            ps_ring_cm.__exit__(None, None, None)

            # pool stack: cx (outer), op, mega (closes first, at tail)
            ps_cx_cm = tc.tile_pool(name="ps_cx", bufs=1, space="PSUM")
            ps_cx = ps_cx_cm.__enter__()
            for h in range(HPC):
                ctx_ps[h] = ps_cx.tile(
                    [DV + 1, CW], F32, tag=f"cx{h}", name=f"cx{h}"
                )
            ps_op_cm = tc.tile_pool(name="ps_op", bufs=1, space="PSUM")
            ps_op = ps_op_cm.__enter__()
            ps_mega_cm = tc.tile_pool(name="ps_mega", bufs=1, space="PSUM")
            ps_mega = ps_mega_cm.__enter__()
            ps_tp1_cm = tc.tile_pool(name="ps_tp1", bufs=1, space="PSUM")
            ps_tp1 = ps_tp1_cm.__enter__()
            tp_t = ps_tp1.tile([P, 2, P], BF16, tag="tp1", name="tp1")

            for tt in range(TT):
                scores_tt(1, tt, ps_mega, 2)
                if tt < 8:
                    transp(8 + tt)
                    ctx_op(0, 2 * tt)
                    ctx_op(0, 2 * tt + 1)
                else:
                    if tt == 8:
                        ps_tp1_cm.__exit__(None, None, None)
                        normalize(0)
                        cc_stage(0)
                    ctx_op(1, tt - 8)

            # ================= chunk 2 =================
            for tt in range(TT):
                scores_tt(2, tt, ps_mega, 2)
                if tt < 8:
                    ctx_op(1, tt + 8)
                else:
                    if tt == 8:
                        normalize(1)
                        cc_stage(1)
                    ctx_op(2, tt - 8)

            # ================= chunk 3 =================
            # ctx(3) at lag 2 (2 k/tt) so normalize(3) follows the last
            # scores almost immediately.
            for tt in range(TT):
                scores_tt(3, tt, ps_mega, 2)
                if tt < 8:
                    ctx_op(2, tt + 8)
                else:
                    if tt == 8:
                        normalize(2)
                        cc_stage(2)
                    ctx_op(3, 2 * (tt - 8))
                    ctx_op(3, 2 * (tt - 8) + 1)

            # ================= tail =================
            ps_mega_cm.__exit__(None, None, None)
            ps_opm_cm = tc.tile_pool(name="ps_opm", bufs=1, space="PSUM")
            ps_opm = ps_opm_cm.__enter__()
            normalize(3)
            cc_stage(3)
            # catin fetches on GpSimd strictly after every trigger; each
            # waits only for its own collective's completion.
            for ci in range(4):
                cc_fetch(ci)
            # op0/op1 (data long since landed) fill the AG2/AG3 mesh
            # windows; op2/op3 follow their fetches.
            for ci in range(4):
                outproj_start(ci)
                for kt in range(KT_D):
                    outproj_mm(ci, kt)
                outproj_store(ci)

            ps_opm_cm.__exit__(None, None, None)
            ps_op_cm.__exit__(None, None, None)
            ps_cx_cm.__exit__(None, None, None)

    nc.compile()
    return nc


def kernel(
    encodings_for_q,
    encodings_for_k,
    encodings_for_v,
    W_q,
    W_k,
    W_v,
    W_out,
    _trace: bool = False,
):
    encodings_for_q = np.asarray(encodings_for_q, dtype=np.float32)
    encodings_for_k = np.asarray(encodings_for_k, dtype=np.float32)
    encodings_for_v = np.asarray(encodings_for_v, dtype=np.float32)
    W_q = np.asarray(W_q, dtype=np.float32)
    W_k = np.asarray(W_k, dtype=np.float32)
    W_v = np.asarray(W_v, dtype=np.float32)
    W_out = np.asarray(W_out, dtype=np.float32)

    if "nc" not in _cache:
        _cache["nc"] = build()
    nc = _cache["nc"]

    eqT = np.ascontiguousarray(encodings_for_q.T).astype(ml_dtypes.bfloat16)
    ekT = np.ascontiguousarray(encodings_for_k.T).astype(ml_dtypes.bfloat16)
    evT = np.ascontiguousarray(encodings_for_v.T).astype(ml_dtypes.bfloat16)

    # pack in SBUF tile order (contiguous per-partition DMA slices):
    # ekp[p, sc4, kt, m], eqp[p, ci, kt, m], evp[p, half, dt, s]
    ekp = np.ascontiguousarray(
        ekT.reshape(KT_D, P, 4, NQ).transpose(1, 2, 0, 3).reshape(P, -1)
    )
    eqp = np.ascontiguousarray(
        eqT.reshape(KT_D, P, NCH, CW).transpose(1, 2, 0, 3).reshape(P, -1)
    )
    evp = np.ascontiguousarray(
        evT.reshape(2, 4, P, S).transpose(2, 0, 1, 3).reshape(P, -1)
    )

    in_maps = []
    for c in range(NCORES):
        hs = slice(HPC * c, HPC * (c + 1))
        in_maps.append(
            {
                "ekp": ekp,
                "eqp": eqp,
                "evp": evp,
                "wq": _prep_w(np.transpose(W_q[hs], (1, 0, 2)).reshape(D, FW)),
                "wk": _prep_w(np.transpose(W_k[hs], (1, 0, 2)).reshape(D, FW)),
                "wv": _prep_w(np.transpose(W_v[hs], (1, 0, 2)).reshape(D, FW)),
                "wo": _prep_w(W_out[:, FW * c : FW * (c + 1)]),
            }
        )

    r = run_bass_kernel_spmd(
        nc, in_maps, core_ids=list(range(NCORES)), trace=_trace
    )
    out = np.concatenate(
        [r.results[c]["outT"].T for c in range(NCORES)], axis=1
    )
    if _trace:
        kernel.last_exec_time_ns = r.exec_time_ns
        kernel.last_insts = (
            r.instructions_and_trace[0] if r.instructions_and_trace else None
        )
    return out.astype(np.float32)
